# revision 4
# baseline (speedup 1.0000x reference)
"""DRN layer kernel for 8 TRN2 NeuronCores (v2).

Math (reference):
    T[j,k,l,m]   = exp(-w[j,k] * (s0[m]-s1[l])^2)
    Pw[i,j,k,l]  = sum_m T[j,k,l,m] * P[i,k,m]
    logsum[i,j,l]= sum_k log(Pw[i,j,k,l])
    out          = softmax_l(logsum + exponent_B[j,l])

Key identity: with P' = P/S (S = sum_m P) and t' = T - 1,
    log Pw = log S + log1p(r),   r = sum_m t'[j,k,l,m] P'[i,k,m]
log S is constant along l so it cancels in the softmax; |r| <= 0.105 so
everything stays in a tiny, well-conditioned range and no max-shift is
needed before the final exp.

Sharding: tensor-parallel over n_upper: 8 cores x 8 upper nodes, full
batch per core. Per core, 64 k-tiles of r land in PSUM (fp32, exact);
each tile is consumed ONCE by one of three routes:
  c) DVE fused chain   chain = (r + 1) * chain   (scalar_tensor_tensor)
  p) ScalarE log1p(r) -> bf16, then PE identity-matmuls accumulate the
     log tile into a PSUM accumulator (fp32, has_written accumulate)
  g) ScalarE log1p(r) -> f32, GpSimd adds into an SBUF accumulator
The chains get log1p'd at the end and fed through route p; exponent_B
seeds the PSUM accumulator via the start=True identity matmul.
"""

import numpy as np

B, NU, NL, QU, QL = 256, 64, 64, 64, 64
NCORES = 8
JLOC = NU // NCORES  # 8 upper nodes per core
JL = JLOC * QU       # 512 = packed (j, l) free dim
KDIM = QL            # 64 contraction rows (m only; no ones/S rows)
PWK = B + JL         # 768 packed width per k: [P'^T (256 i) | t' (512 jl)]
NKB = NL // 2        # 32 two-k DMA blocks

# route per k: c0/c1 = DVE product chains, p = ScalarE->PE-accumulate,
# g = ScalarE->GpSimd accumulate. Balanced for engine busy-time (DVE
# ~1.19us/tile, Sc ~1.0, GpS ~2.9, PE add-mm ~0.53); g ends early (Q7
# pipeline drain is slow) and the tail is p so the finish is fast.
def _make_route(nc_=30, np_=21, ng_=13):
    # largest-remainder interleave of c/p/g; g confined to [2, 52] and
    # the last 3 ks forced to p
    route = ["p"] * NL
    gpos = [2 + round(i * 50 / (ng_ - 1)) for i in range(ng_)]
    rest = [k for k in range(NL - 3) if k not in gpos]
    credit = 0.0
    ci = 0
    ncr, npr = nc_, np_ - 3
    for k in rest:
        credit += ncr / (ncr + npr) if (ncr + npr) else 0
        if credit >= 1.0 and ncr > 0:
            credit -= 1.0
            route[k] = f"c{ci}"
            ci ^= 1
            ncr -= 1
        else:
            route[k] = "p"
            npr -= 1
    for k in gpos:
        route[k] = "g"
    return route


ROUTE = _make_route()
assert len(ROUTE) == NL


def _build_program():
    import concourse.bass as bass
    import concourse.bacc as bacc
    import concourse.mybir as mybir
    from concourse.tile import TileContext

    f32 = mybir.dt.float32
    bf16 = mybir.dt.bfloat16
    AF = mybir.ActivationFunctionType
    ALU = mybir.AluOpType

    nc = bacc.Bacc(None, target_bir_lowering=False)
    PTT = nc.declare_dram_parameter("PTT", [NKB, KDIM, 2 * PWK], bf16,
                                    isOutput=False)
    EB = nc.declare_dram_parameter("EB", [128, 2 * JL], bf16, isOutput=False)
    IDB = nc.declare_dram_parameter("IDB", [128, 128], bf16, isOutput=False)
    IDF = nc.declare_dram_parameter("IDF", [128, 128], f32, isOutput=False)
    OUT = nc.declare_dram_parameter("out", [2, 128, JL], f32, isOutput=True)

    with TileContext(nc) as tc:
        with (
            tc.tile_pool(name="ptt", bufs=6) as ppool,
            tc.tile_pool(name="cst", bufs=1) as cpool,
            tc.tile_pool(name="ps", bufs=3, space="PSUM") as pspool,
            tc.tile_pool(name="acc", bufs=1, space="PSUM") as apool,
            tc.tile_pool(name="lgb", bufs=6) as lbpool,
            tc.tile_pool(name="lgf", bufs=3) as lfpool,
            tc.tile_pool(name="ch", bufs=1) as chpool,
            tc.tile_pool(name="sm", bufs=2) as smpool,
            tc.tile_pool(name="ot", bufs=2) as opool,
        ):
            ebt = cpool.tile([128, 2 * JL], bf16, tag="ebt")
            nc.sync.dma_start(out=ebt[:], in_=EB[:, :])
            idb = cpool.tile([128, 128], bf16, tag="idb")
            nc.sync.dma_start(out=idb[:], in_=IDB[:, :])
            idf = cpool.tile([128, 128], f32, tag="idf")
            nc.sync.dma_start(out=idf[:], in_=IDF[:, :])

            acc_ps = apool.tile([128, 2 * JL], f32, tag="accps", name="accps")
            # seed the accumulator with exponent_B; start=True clears the
            # banks' has_written bits so later identity-matmuls accumulate
            for h in range(2):
                nc.tensor.matmul(acc_ps[:, h * JL:(h + 1) * JL],
                                 lhsT=idb[:], rhs=ebt[:, h * JL:(h + 1) * JL],
                                 start=True, stop=False)

            acc_g = chpool.tile([128, 2 * JL], f32, tag="accg", name="accg")
            chains = {
                "c0": chpool.tile([128, 2 * JL], f32, tag="ch0", name="ch0"),
                "c1": chpool.tile([128, 2 * JL], f32, tag="ch1", name="ch1"),
            }
            started = {"c0": False, "c1": False, "g": False}

            # deferred PE accumulate-matmuls: emit with a lag of a few ks
            # so the PE FIFO never waits on a ScalarE log in flight
            pending = []

            def flush_addmm(n):
                for _ in range(min(n, len(pending))):
                    lg, last = pending.pop(0)
                    ident = idb if lg.dtype == bf16 else idf
                    for h in range(2):
                        nc.tensor.matmul(
                            acc_ps[:, h * JL:(h + 1) * JL],
                            lhsT=ident[:],
                            rhs=lg[:, h * JL:(h + 1) * JL],
                            start=False, stop=last and h == 1)

            for kb in range(NKB):
                ptt = ppool.tile([KDIM, 2 * PWK], bf16, tag="ptt")
                nc.sync.dma_start(out=ptt[:], in_=PTT[kb])
                for kk in range(2):
                    k = 2 * kb + kk
                    off = kk * PWK
                    ps = pspool.tile([128, 2 * JL], f32, tag="ps", name="ps")
                    for ih in range(2):
                        nc.tensor.matmul(
                            ps[:, ih * JL:(ih + 1) * JL],
                            lhsT=ptt[:, off + ih * 128:off + (ih + 1) * 128],
                            rhs=ptt[:, off + B:off + PWK],
                            start=True, stop=True)
                    r = ROUTE[k]
                    if r in ("c0", "c1"):
                        ch = chains[r]
                        if not started[r]:
                            nc.vector.tensor_scalar_add(ch[:], ps[:], 1.0)
                            started[r] = True
                        else:
                            nc.vector.scalar_tensor_tensor(
                                ch[:], ps[:], 1.0, ch[:],
                                op0=ALU.add, op1=ALU.mult)
                    elif r == "p":
                        lgb = lbpool.tile([128, 2 * JL], bf16, tag="lgb",
                                          name="lgb")
                        nc.scalar.activation(lgb[:], ps[:], AF.Ln, bias=1.0)
                        pending.append((lgb, False))
                    else:  # g
                        lgf = lfpool.tile([128, 2 * JL], f32, tag="lgf",
                                          name="lgf")
                        nc.scalar.activation(lgf[:], ps[:], AF.Ln, bias=1.0)
                        if not started["g"]:
                            nc.gpsimd.tensor_copy(acc_g[:], lgf[:])
                            started["g"] = True
                        else:
                            nc.gpsimd.tensor_add(acc_g[:], acc_g[:], lgf[:])
                # keep ~2 blocks of lag before accumulating a log tile
                if kb >= 2:
                    flush_addmm(1)

            # chains -> log1p -> PE accumulate; GpSimd accumulator last
            for cname in ("c0", "c1"):
                clg = lbpool.tile([128, 2 * JL], bf16, tag="clg",
                                  name=f"clg{cname}")
                nc.scalar.activation(clg[:], chains[cname][:], AF.Ln)
                pending.append((clg, False))
            pending.append((acc_g, True))
            flush_addmm(len(pending))

            # softmax over l: logits are centered (log S dropped), so no
            # max-shift is needed before exp
            NG = 2 * JLOC  # 16 (ih, j) groups
            exs = opool.tile([128, 2 * JL], f32, tag="exs")
            nc.scalar.activation(exs[:], acc_ps[:], AF.Exp)
            exs3 = exs[:, :].rearrange("p (g l) -> p g l", g=NG)
            smb = smpool.tile([128, NG], f32, tag="smb")
            nc.vector.tensor_reduce(
                smb[:], exs3, axis=mybir.AxisListType.X, op=ALU.add)
            rcb = smpool.tile([128, NG], f32, tag="rcb")
            nc.vector.reciprocal(rcb[:], smb[:])
            ot = opool.tile([128, 2 * JL], f32, tag="otb", name="otb")
            ot3 = ot[:, :].rearrange("p (g l) -> p g l", g=NG)
            nc.vector.tensor_mul(
                ot3, exs3, rcb[:, :].broadcast_to((128, NG, QU)))
            for ih in range(2):
                nc.sync.dma_start(out=OUT[ih, :, :],
                                  in_=ot[:, ih * JL:(ih + 1) * JL])
    nc.compile()
    return nc


def _host_prep(P, weight, bias_abs, bias_q, lambda_abs, lambda_q):
    """Per-core input maps. Host does only O(weights) work plus linear
    passes over P (sum, normalize, transpose, cast)."""
    import ml_dtypes

    bf16 = ml_dtypes.bfloat16
    s1 = np.arange(QU, dtype=np.float64) / QU
    s0 = np.arange(QL, dtype=np.float64) / QL
    diff2 = (s0[None, :] - s1[:, None]) ** 2            # [l, m]
    # t' = T - 1 = expm1(-w * diff2): [NU, NL, QU(l), QL(m)]
    t_full = np.expm1(-weight[:, :, None, None].astype(np.float64)
                      * diff2[None, None, :, :]).astype(np.float32)
    sq = s1
    expB = (-bias_q.astype(np.float64) * (sq[None, :] - lambda_q) ** 2
            - bias_abs.astype(np.float64)
            * np.abs(sq[None, :] - lambda_abs)).astype(np.float32)

    P32 = P.astype(np.float32)
    S = P32.sum(axis=2, dtype=np.float64)               # [i, k]
    Pn = (P32 / S[:, :, None]).astype(np.float32)       # P' = P/S
    PT_bf = Pn.transpose(1, 2, 0).astype(bf16)          # [k, m, i]

    ident = np.eye(128, dtype=np.float32)
    identb = ident.astype(bf16)

    in_maps = []
    for c in range(NCORES):
        tc_ = t_full[c * JLOC:(c + 1) * JLOC]           # [8, k, l, m]
        tc_ = tc_.transpose(1, 3, 0, 2).reshape(NL, QL, JL)  # [k, m, (j,l)]
        PTTc = np.empty((NL, KDIM, PWK), dtype=bf16)
        PTTc[:, :, :B] = PT_bf
        PTTc[:, :, B:] = tc_.astype(bf16)
        PTTc = np.ascontiguousarray(
            PTTc.reshape(NKB, 2, KDIM, PWK).transpose(0, 2, 1, 3)
            .reshape(NKB, KDIM, 2 * PWK))
        eb_row = np.tile(expB[c * JLOC:(c + 1) * JLOC].reshape(JL), 2)
        EBc = np.ascontiguousarray(
            np.broadcast_to(eb_row, (128, 2 * JL)).astype(bf16))
        in_maps.append({"PTT": PTTc, "EB": EBc, "IDB": identb, "IDF": ident})
    return in_maps


_PROGRAM = None


def _get_program():
    global _PROGRAM
    if _PROGRAM is None:
        _PROGRAM = _build_program()
    return _PROGRAM


def run_on_device(in_maps, trace=False):
    from concourse.bass_utils import run_bass_kernel_spmd
    nc = _get_program()
    return run_bass_kernel_spmd(
        nc, in_maps, core_ids=list(range(NCORES)), trace=trace,
    )


def assemble(results):
    out = np.empty((B, NU, QU), dtype=np.float32)
    for c in range(NCORES):
        rc = results[c]["out"].reshape(B, JLOC, QU)
        out[:, c * JLOC:(c + 1) * JLOC, :] = rc
    return out


def kernel(P, weight, bias_abs, bias_q, lambda_abs, lambda_q):
    in_maps = _host_prep(P, weight, bias_abs, bias_q, lambda_abs, lambda_q)
    res = run_on_device(in_maps, trace=False)
    return assemble(res.results)
